# revision 9
# baseline (speedup 1.0000x reference)
"""Trainium2 Bass kernel: LayerNorm -> QKV -> linear (elu+1) attention -> proj.

Data-parallel over batch: 8 batch elements, one per NeuronCore. All matmuls
in bf16 (fp32 accumulation in PSUM).

Phase-1 restructure vs. the bounce-transpose baseline: the RAW (un-normalized)
x is DMA-transposed straight from input DRAM (dep-free, prefetchable from
t=0), and LayerNorm is applied algebraically around the matmul:

    qkv[t,e] = rstd[t] * (x @ Wg^T)[t,e] - rstd[t]*mu[t] * c[e],
    c[e] = sum_d Wg[e,d]

 - the rstd scale is folded into the transposed x (xs = xT * A, A = rstd
   broadcast across partitions, computed on the otherwise-idle GPSIMD engine)
 - the q-side rank-1 term is applied during PSUM evacuation with a
   B2 = -rstd*mu broadcast row-tile and a per-partition c_q scalar
 - the k-side term uses a C_k column-broadcast tile and per-partition -rstd*mu
 - the v-side term is folded into the kv accumulation as an extra accumulated
   column (content -rstd*mu), fixed up once on the tiny kv matrix
 - row-major (token-order) rstd/-rstd*mu rows are produced via a PE transpose
   + small DRAM bounce + stride-0 broadcast reads

Phase 2: attention matmuls use a block-diagonal [128,128] stationary (both
head parities in one pass); z replication uses 2 (not 12) broadcast DMAs.

Self-contained: hardcodes shapes from the problem spec.
"""

import numpy as np
import ml_dtypes

from concourse import bass, bacc, tile, mybir
from concourse.bass import ts, ds
from concourse.bass_utils import run_bass_kernel_spmd

F32 = mybir.dt.float32
BF16 = mybir.dt.bfloat16
AF = mybir.ActivationFunctionType
ALU = mybir.AluOpType

# Problem shapes
N = 4096          # tokens per batch element
D = 768           # model dim
H = 12            # heads
HD = 64           # head dim
E3 = 3 * D        # qkv width
P = 128
KT = D // P       # 6 d-tiles
NT = N // P       # 32 token tiles
CH = 8            # token chunks of 512
TPC = NT // CH    # 4 token tiles per chunk
CW = N // CH      # 512 chunk width
LN_EPS = 1e-5
EPS = 1e-6
KVW = HD + 2      # kv accum block: 64 kv cols + ksum + (-rstd*mu)-weighted

N_CORES = 8
LDW_SKIP = True


def _build(with_qkv_bias: bool, dbg: bool = False):
    """Build the single-core program (SPMD: same NEFF on all 8 cores)."""
    nc = bacc.Bacc("TRN2", target_bir_lowering=False, debug=False,
                   num_devices=N_CORES)

    x_d = nc.dram_tensor("x", [N, D], BF16, kind="ExternalInput").ap()
    wqkvT_d = nc.dram_tensor("wqkvT", [D, E3], BF16, kind="ExternalInput").ap()
    wprojT_d = nc.dram_tensor("wprojT", [D, D], BF16, kind="ExternalInput").ap()
    bpack_d = nc.dram_tensor("bpack", [1, P + D], F32, kind="ExternalInput").ap()
    cq_d = nc.dram_tensor("cq", [P, KT], F32, kind="ExternalInput").ap()
    cbf_d = nc.dram_tensor("cbf", [1, 2 * D], BF16, kind="ExternalInput").ap()
    out_d = nc.dram_tensor("out", [N, D], BF16, kind="ExternalOutput").ap()

    from contextlib import ExitStack
    with tile.TileContext(nc) as tc, ExitStack() as stk:
        _kernel(tc, stk, nc, x_d, wqkvT_d, wprojT_d, bpack_d, cq_d, cbf_d,
                out_d, dbg)

    nc.compile()
    return nc


def _kernel(tc, stk, nc, x_d, wqkvT_d, wprojT_d, bpack_d, cq_d, cbf_d,
            out_d, dbg=False):
    def dump(name, tl, shape, dtype):
        if not dbg:
            return
        d = nc.dram_tensor("dbg_" + name, shape, dtype, kind="ExternalOutput").ap()
        nc.sync.dma_start(d, tl)

    from contextlib import ExitStack, nullcontext
    consts = stk.enter_context(tc.tile_pool(name="consts", bufs=1))
    stk1 = stk.enter_context(ExitStack())
    ppersist = stk1.enter_context(tc.tile_pool(name="ppersist", bufs=1, space="PSUM"))

    # DMA queue split (HWDGE queues are in-order):
    #   SYNC queue: xT transposes (dep-free, issued earliest) + x [t,d] tile
    #               loads (stats only), phase-2 zb/zr.
    #   ACT queue:  weights (q columns first), per-piece stat-row bounce +
    #               A/B2 broadcast reads, out stores.
    x_prefetch = {}
    xT_pref = {}
    xTp = stk1.enter_context(tc.tile_pool(name="xT", bufs=6))
    xsp = stk1.enter_context(tc.tile_pool(name="xs", bufs=5))
    xpool = stk1.enter_context(tc.tile_pool(name="x", bufs=10))

    def load_x_chunk(c0):
        for t in range(c0 * TPC, (c0 + 1) * TPC):
            xt = xpool.tile([P, D], BF16)
            nc.sync.dma_start(xt[:], x_d[ts(t, P), :])
            x_prefetch[t] = xt

    def load_xT(c, pieces=1):
        """DMA-transpose raw x for chunk c straight from input DRAM."""
        t3 = [xTp.tile([P, 3, CW], BF16, tag="xT3", name=f"xT3_{c}_{h}")
              for h in range(2)]
        w = CW // pieces
        for pp in range(pieces):
            for h in range(2):
                nc.sync.dma_start_transpose(
                    out=t3[h][:, :, ds(pp * w, w)],
                    in_=x_d[ds(c * CW + pp * w, w), ds(h * 384, 384)])
        xT_pref[c] = t3

    # earliest possible: transposes for chunks 0..1 + x tiles for stats
    load_xT(0, pieces=2)
    load_x_chunk(0)
    load_xT(1)
    load_x_chunk(1)

    wqkvT = consts.tile([P, KT, E3], BF16)
    wprojT = consts.tile([P, KT, D], BF16)
    wq_r = wqkvT_d.rearrange("(kt p) e -> p kt e", p=P)
    wp_r = wprojT_d.rearrange("(kt p) e -> p kt e", p=P)
    bproj_row = consts.tile([1, D], F32)
    nc.scalar.dma_start(bproj_row[:], bpack_d[:, P:P + D])
    # q weight columns first (first consumers)
    for kt in range(KT):
        nc.scalar.dma_start(wqkvT[:, kt, 0:D], wq_r[:, kt, 0:D])
    # small constant tiles
    cq_sb = consts.tile([P, KT], F32)
    nc.scalar.dma_start(cq_sb[:], cq_d[:, :])
    C_k = consts.tile([P, D], BF16)
    nc.scalar.dma_start(C_k[:], cbf_d[0:1, 0:D].broadcast_to([P, D]))
    Cv = consts.tile([P, KT, HD], BF16)
    cv_r = cbf_d[0:1, D:2 * D].rearrange("a (pp s e) -> a s pp e", s=2, e=HD)
    for s_ in range(2):
        nc.scalar.dma_start(Cv[ds(64 * s_, 64), :, :],
                            cv_r[0:1, s_].broadcast_to([64, KT, HD]))

    ones_row = consts.tile([1, P], F32)
    nc.vector.memset(ones_row[:], 1.0)
    bias_sb = consts.tile([P, D], F32)
    ones12 = consts.tile([P, H], BF16)
    nc.vector.memset(ones12[:], 1.0)

    # zero-row for psum-bank init matmuls
    zrow = consts.tile([1, 512], BF16)
    nc.vector.memset(zrow[:], 0.0)
    ones_bf = consts.tile([1, P], BF16)
    nc.vector.memset(ones_bf[:], 1.0)

    # --- kv accumulator ---
    # pair p = h//2 -> cols [66p, 66p+66), head parity s=h%2 -> partitions
    # [64s, 64s+64). col 64 = k_sum, col 65 = sum_t k*(-rstd*mu) (v fixup).
    kv_ps = ppersist.tile([P, KT * KVW], F32)
    nc.tensor.matmul(kv_ps[:], ones_bf[:], zrow[:, 0:KT * KVW], start=True,
                     stop=False, skip_group_check=True)

    stat = stk1.enter_context(tc.tile_pool(name="stat", bufs=12))
    abp = stk1.enter_context(tc.tile_pool(name="ab", bufs=4))
    kvps = stk1.enter_context(tc.tile_pool(name="kvps", bufs=2, space="PSUM"))
    qpsp = stk1.enter_context(tc.tile_pool(name="qpsp", bufs=1, space="PSUM"))
    evac = stk1.enter_context(tc.tile_pool(name="evac", bufs=4))
    upool = stk1.enter_context(tc.tile_pool(name="u", bufs=3))

    qT_all = consts.tile([P, KT, N], BF16)
    dramp = stk.enter_context(tc.tile_pool(name="dram", bufs=6, space="DRAM"))

    # ============ PHASE 1 ================================================
    def process_piece(c, tt0, ntt, order):
        W = ntt * P
        base = c * CW + tt0 * P
        I32 = mybir.dt.int32
        xT3 = xT_pref[c]
        prio = tc.high_priority() if (c == 0 and tt0 == 0) else nullcontext()
        with prio:
            # LayerNorm stats (fp32), per token tile of the piece
            mv_all = stat.tile([P, TPC, 2], F32, tag="mv")
            for j in range(ntt):
                xt = x_prefetch.pop(c * TPC + tt0 + j)
                st6 = stat.tile([P, 2, 6], F32)
                nc.vector.bn_stats(st6[:, 0], xt[:, 0:D // 2])
                nc.vector.bn_stats(st6[:, 1], xt[:, D // 2:D])
                nc.vector.bn_aggr(mv_all[:, j], st6[:])
            # batched rstd = rsqrt(var+eps): bit-trick seed + 1 Newton step
            veps = stat.tile([P, TPC], F32)
            nc.vector.tensor_scalar_add(veps[:, 0:ntt], mv_all[:, 0:ntt, 1],
                                        LN_EPS)
            t1 = stat.tile([P, TPC], I32, tag="rs_t1")
            nc.vector.tensor_scalar(t1[:, 0:ntt],
                                    veps[:, 0:ntt].bitcast(I32), 1, None,
                                    op0=ALU.arith_shift_right)
            rstd = stat.tile([P, TPC], F32)
            nc.vector.tensor_scalar(rstd[:, 0:ntt].bitcast(I32), t1[:, 0:ntt],
                                    -1, 0x5F3759DF, op0=ALU.mult, op1=ALU.add)
            a = stat.tile([P, TPC], F32, tag="rs_a")
            nc.vector.tensor_tensor(a[:, 0:ntt], rstd[:, 0:ntt], rstd[:, 0:ntt],
                                    ALU.mult)
            nc.vector.tensor_tensor(a[:, 0:ntt], a[:, 0:ntt], veps[:, 0:ntt],
                                    ALU.mult)
            nc.vector.tensor_scalar(a[:, 0:ntt], a[:, 0:ntt], -0.5, 1.5,
                                    op0=ALU.mult, op1=ALU.add)
            nc.vector.tensor_tensor(rstd[:, 0:ntt], rstd[:, 0:ntt], a[:, 0:ntt],
                                    ALU.mult)
            # m2 = -rstd*mu (fp32, per-partition scalar use)
            m2f = stat.tile([P, TPC], F32, tag="m2f")
            nc.vector.scalar_tensor_tensor(m2f[:, 0:ntt], rstd[:, 0:ntt], -1.0,
                                           mv_all[:, 0:ntt, 0],
                                           op0=ALU.mult, op1=ALU.mult)
            # [rstd | m2] -> DRAM bounce; the broadcast READ transposes to
            # token order via a swapped-AP pattern (8B-granular strided src).
            stat8 = stat.tile([P, 2 * ntt], BF16, tag="s8")
            nc.vector.tensor_copy(stat8[:, 0:ntt], rstd[:, 0:ntt])
            nc.vector.tensor_copy(stat8[:, ntt:2 * ntt], m2f[:, 0:ntt])
        rs_d = dramp.tile([P, 2 * TPC], BF16)
        nc.scalar.dma_start(rs_d[:, 0:2 * ntt], stat8[:])
        A_t = abp.tile([P, CW], BF16, tag="A")
        B2_t = abp.tile([P, CW], BF16, tag="B2")
        for j in range(ntt):
            nc.scalar.dma_start(
                A_t[:, ds(j * P, P)],
                rs_d[:, j:j + 1].rearrange("p o -> o p").broadcast_to([P, P]))
            nc.scalar.dma_start(
                B2_t[:, ds(j * P, P)],
                rs_d[:, ntt + j:ntt + j + 1].rearrange("p o -> o p")
                .broadcast_to([P, P]))

        # xs = xT * rstd  (GPSIMD; raw transposed x -> rstd-scaled)
        xs3 = [xsp.tile([P, 3, CW], BF16, tag="xs3", name=f"xs3_{c}_{tt0}_{h}")
               for h in range(2)]
        for h in range(2):
            for k3 in range(3):
                nc.gpsimd.tensor_tensor(xs3[h][:, k3, 0:W],
                                        xT3[h][:, k3, ds(tt0 * P, W)],
                                        A_t[:, 0:W], ALU.mult)
        xs = [xs3[kt // 3][:, kt % 3, 0:W] for kt in range(KT)]

        def q_chain(m):
            q_ps = qpsp.tile([P, 512], F32, tag="qps1")
            for kt in range(KT):
                nc.tensor.matmul(q_ps[:, 0:W], wqkvT[:, kt, ts(m, P)], xs[kt],
                                 start=(kt == 0), stop=(kt == KT - 1))
            # u = q_ps + B2*cq[m]  (rank-1 LN mean correction)
            u = upool.tile([P, CW], F32, tag="qu")
            nc.vector.scalar_tensor_tensor(u[:, 0:W], B2_t[:, 0:W],
                                           cq_sb[:, m:m + 1], q_ps[:, 0:W],
                                           op0=ALU.mult, op1=ALU.add)
            # elu1(u) = min(exp(u),1) + relu(u)
            et = evac.tile([P, CW], BF16, tag="elu_e")
            nc.scalar.activation(et[:, 0:W], u[:, 0:W], AF.Exp)
            rt = evac.tile([P, CW], BF16, tag="elu_r")
            nc.scalar.activation(rt[:, 0:W], u[:, 0:W], AF.Relu)
            nc.vector.scalar_tensor_tensor(qT_all[:, m, ds(base, W)],
                                           et[:, 0:W], 1.0, rt[:, 0:W],
                                           op0=ALU.min, op1=ALU.add)

        def kv_chain(j):
            t = c * TPC + tt0 + j
            kv3 = kvps.tile([P, 3 * 512], F32, tag="ph1ps")
            for kt in range(KT):
                for jj in range(3):
                    mm = nc.tensor.matmul(
                        kv3[:, ts(jj, 512)],
                        xs[kt][:, ts(j, P)],
                        wqkvT[:, kt, ds(D + jj * 512, 512)],
                        start=(kt == 0), stop=(kt == KT - 1))
                    if jj > 0 and LDW_SKIP:
                        mm.ldweights = False  # same stationary as jj-1
            # k = elu1(kv3[:, 0:D] + m2*C_k)
            uk = upool.tile([P, D], F32, tag="uk")
            nc.vector.scalar_tensor_tensor(uk[:], C_k[:], m2f[:, j:j + 1],
                                           kv3[:, 0:D],
                                           op0=ALU.mult, op1=ALU.add)
            ek = evac.tile([P, D], BF16, tag="elu_ek")
            nc.scalar.activation(ek[:], uk[:], AF.Exp)
            rk = evac.tile([P, D], BF16, tag="elu_rk")
            nc.vector.tensor_scalar_max(rk[:], uk[:], 0.0)
            ktile = evac.tile([P, D], BF16, tag="ktile")
            nc.vector.scalar_tensor_tensor(ktile[:], ek[:], 1.0, rk[:],
                                           op0=ALU.min, op1=ALU.add)
            # v' = [v_h | 1 | m2] per head: [128, 12, 66]
            vtile = evac.tile([P, H, KVW], BF16, tag="vtile")
            nc.vector.memset(vtile[:, :, HD:HD + 1], 1.0)
            nc.vector.tensor_scalar(vtile[:, :, HD + 1], ones12[:],
                                    m2f[:, j:j + 1], None, op0=ALU.mult)
            nc.scalar.activation(
                vtile[:, :, 0:HD],
                kv3[:, D:2 * D].rearrange("p (h e) -> p h e", h=H),
                AF.Copy)
            # kv accumulation: 12 heads, 2 packed per psum column block
            for h in range(H):
                p_, s_ = h // 2, h % 2
                nc.tensor.matmul(
                    kv_ps[ds(64 * s_, 64), ds(KVW * p_, KVW)],
                    ktile[:, ds(HD * h, HD)],
                    vtile[:, h],
                    start=False, stop=(t == NT - 1),
                    skip_group_check=True,
                    tile_position=(0, 64 * s_))

        if order == "q_first":
            for m in range(KT):
                q_chain(m)
            for j in range(ntt):
                kv_chain(j)
        elif order == "kv_first":
            for j in range(ntt):
                kv_chain(j)
            for m in range(KT):
                q_chain(m)
        else:
            for j in range(ntt):
                q_chain(j)
                kv_chain(j)
            for m in range(ntt, KT):
                q_chain(m)

    # kv-weight columns land after chunk-0 piece-0's stat bounce on the ACT
    # queue; interleaved so neither path starves.
    for c in range(CH):
        if c == 0:
            process_piece(0, 0, 2, "q_first")
            # kv columns 1/3 (interleave with piece stat DMAs on ACT queue)
            for kt in range(KT):
                nc.scalar.dma_start(wqkvT[:, kt, D:D + 512],
                                    wq_r[:, kt, D:D + 512])
            process_piece(0, 2, 2, "q_first")
            for kt in range(KT):
                nc.scalar.dma_start(wqkvT[:, kt, D + 512:D + 1024],
                                    wq_r[:, kt, D + 512:D + 1024])
            for kt in range(KT):
                nc.scalar.dma_start(wqkvT[:, kt, D + 1024:E3],
                                    wq_r[:, kt, D + 1024:E3])
        elif c == CH - 1:
            process_piece(c, 0, TPC, "kv_first")
        else:
            process_piece(c, 0, TPC, "interleave")
        # prefetch for chunk c+2: transposes first (PE-critical), then x tiles
        if c + 2 < CH:
            load_xT(c + 2)
            load_x_chunk(c + 2)
        if c == 4:
            for kt in range(KT):
                nc.scalar.dma_start(wprojT[:, kt], wp_r[:, kt])

    # ================= PHASE 1.5: kv fixup -> block-diag, ksel ===========
    # kvbd[:, p] is a [128,128] block-diagonal stationary: parity-0 head's
    # kv in the (0:64,0:64) block, parity-1 in (64:128,64:128). The v-side
    # LN-mean fixup (+ accumulated col65 * c_v) is applied during this evac.
    kvbd = consts.tile([P, KT, P], BF16)
    nc.vector.memset(kvbd[:], 0.0)
    kwsb = consts.tile([P, KT], F32)
    kv_g = kv_ps[:].rearrange("p (g w) -> p g w", w=KVW)
    nc.scalar.activation(kwsb[:], kv_g[:, :, HD + 1], AF.Copy)
    for p_ in range(KT):
        for s_ in range(2):
            nc.vector.scalar_tensor_tensor(
                kvbd[ds(64 * s_, 64), p_, ds(64 * s_, 64)],
                Cv[ds(64 * s_, 64), p_],
                kwsb[ds(64 * s_, 64), p_:p_ + 1],
                kv_ps[ds(64 * s_, 64), ds(KVW * p_, HD)],
                op0=ALU.mult, op1=ALU.add)
    dump("kvbd", kvbd[:], [P, KT, P], BF16)
    dump("qTd", qT_all[:], [P, KT, N], BF16)
    ksel = consts.tile([P, KT, H], BF16)
    nc.vector.memset(ksel[:], 0.0)
    for kt in range(KT):
        for s_ in range(2):
            h = 2 * kt + s_
            nc.vector.tensor_copy(
                ksel[ds(64 * s_, 64), kt, h:h + 1],
                kv_ps[ds(64 * s_, 64), ds(KVW * kt + HD, 1)])

    stk1.close()

    # --- broadcast b_proj to [128, D] fp32 via K=1 fp32 matmuls ---
    with tc.tile_pool(name="pbias", bufs=1, space="PSUM") as pbias:
        for j, w_ in ((0, 512), (1, 256)):
            bias_ps = pbias.tile([P, 512], F32)
            nc.tensor.matmul(bias_ps[:, :w_], ones_row[:],
                             bproj_row[:, ds(j * 512, w_)],
                             start=True, stop=True)
            nc.vector.tensor_copy(bias_sb[:, ds(j * 512, w_)], bias_ps[:, :w_])

    zps = stk.enter_context(tc.tile_pool(name="zps", bufs=2, space="PSUM"))
    atps = stk.enter_context(tc.tile_pool(name="atps", bufs=2, space="PSUM"))
    ops_ = stk.enter_context(tc.tile_pool(name="ops", bufs=2, space="PSUM"))
    ph2 = stk.enter_context(tc.tile_pool(name="ph2", bufs=3))
    zrpool = stk.enter_context(tc.tile_pool(name="zr", bufs=4))

    # ============ PHASE 2: z, attn out, proj ==============================
    def z_chain(c):
        qT = qT_all[:, :, ts(c, CW)]
        z_ps = zps.tile([H, CW], F32)
        for kt in range(KT):
            nc.tensor.matmul(z_ps[:], ksel[:, kt], qT[:, kt],
                             start=(kt == 0), stop=(kt == KT - 1))
        zb = ph2.tile([H, CW], BF16, tag="zb")
        nc.scalar.add_instruction(mybir.InstActivation(
            name=nc.get_next_instruction_name(),
            func=AF.Reciprocal,
            ins=[nc.scalar.lower_ap(z_ps[:]),
                 mybir.ImmediateValue(dtype=F32, value=EPS),
                 mybir.ImmediateValue(dtype=F32, value=1.0),
                 mybir.ImmediateValue(dtype=F32, value=0.0)],
            outs=[nc.scalar.lower_ap(zb[:])]))
        # bounce zb to DRAM, then 2 parity broadcast-reads into [128, KT, CW]
        zb_dram = dramp.tile([H, CW], BF16)
        nc.sync.dma_start(zb_dram[:], zb[:])
        zr = zrpool.tile([P, KT, CW], BF16, tag="zr")
        zb_r = zb_dram.rearrange("(pp s) w -> s pp w", s=2)
        for s_ in range(2):
            nc.sync.dma_start(
                zr[ds(64 * s_, 64), :, :],
                zb_r[s_:s_ + 1].broadcast_to([64, KT, CW]))
        return zr

    def attn_mms(c, fused):
        """attn_T[e, t] per head pair on unscaled qT; one block-diagonal
        [128,128] stationary covers both parities in a single 512-wide pass."""
        qT = qT_all[:, :, ts(c, CW)]
        attnT = ph2.tile([P, KT, CW], BF16, tag="attnT")
        for p_ in range(KT):
            at_ps = atps.tile([P, CW], F32)
            nc.tensor.matmul(at_ps[:], kvbd[:, p_], qT[:, p_],
                             start=True, stop=True)
            if fused:
                nc.vector.tensor_mul(attnT[:, p_], at_ps[:],
                                     zr_tiles[c][:, p_])
            else:
                nc.scalar.activation(attnT[:, p_], at_ps[:], AF.Copy)
        return attnT

    zr_tiles = {}
    attn_tiles = {}
    attn_tiles[0] = attn_mms(0, fused=False)
    attn_tiles[1] = attn_mms(1, fused=False)
    for c0 in range(3):
        zr_tiles[c0] = z_chain(c0)

    for c in range(CH):
        attnT = attn_tiles.pop(c)
        zr = zr_tiles.pop(c)
        if c < 2:
            for kt in range(KT):
                nc.vector.tensor_mul(attnT[:, kt], attnT[:, kt], zr[:, kt])
        if c == 0:
            dump("zr0", zr[:], [P, KT, CW], BF16)
            dump("attnT0", attnT[:], [P, KT, CW], BF16)

        # proj: out[t, e] = sum_d attnT[d, t] * wprojT[d, e]  (+ bias)
        for tt in range(TPC):
            t = c * TPC + tt
            o_ps = ops_.tile([P, D], F32)
            for kt in range(KT):
                for j, w_ in ((0, 512), (1, 256)):
                    mm = nc.tensor.matmul(
                        o_ps[:, ds(j * 512, w_)],
                        attnT[:, kt, ts(tt, P)],
                        wprojT[:, kt, ds(j * 512, w_)],
                        start=(kt == 0), stop=(kt == KT - 1))
                    if j > 0 and LDW_SKIP:
                        mm.ldweights = False  # same stationary as j-1
            osb = ph2.tile([P, D], BF16, tag="osb")
            nc.vector.tensor_tensor(osb[:], o_ps[:], bias_sb[:], ALU.add)
            nc.scalar.dma_start(out_d[ts(t, P), :], osb[:])

        if c + 3 < CH:
            zr_tiles[c + 3] = z_chain(c + 3)
        if c + 2 < CH:
            attn_tiles[c + 2] = attn_mms(c + 2, fused=True)


_CACHE = {}


def _get_nc(with_qkv_bias: bool, dbg: bool = False):
    key = ("nc", with_qkv_bias, dbg)
    if key not in _CACHE:
        _CACHE[key] = _build(with_qkv_bias, dbg)
    return _CACHE[key]


def kernel(x, ln_gamma, ln_beta, w_qkv, w_proj, b_proj, trace=False, dbg=False):
    x = np.asarray(x, dtype=np.float32)
    ln_gamma = np.asarray(ln_gamma, dtype=np.float32)
    ln_beta = np.asarray(ln_beta, dtype=np.float32)
    w_qkv = np.asarray(w_qkv, dtype=np.float32)
    w_proj = np.asarray(w_proj, dtype=np.float32)
    b_proj = np.asarray(b_proj, dtype=np.float32)
    bsz = x.shape[0]
    assert x.shape == (bsz, N, D) and bsz == N_CORES

    # Fold LN affine into the qkv projection (exact algebra):
    #   y = xhat*gamma + beta  =>  qkv = xhat @ (gamma*W)^T + W@beta
    wq_eff = (w_qkv * ln_gamma[None, :])          # [E3, D]
    cqkv = w_qkv @ ln_beta                        # [E3]
    with_bias = bool(np.any(cqkv))
    if with_bias:
        raise NotImplementedError(
            "nonzero W@beta path not wired into the device kernel")

    # c[e] = sum_d Wg[e,d] for the LN mean correction
    crow = wq_eff.sum(axis=1).astype(np.float32)  # [E3]
    cq_hd = np.ascontiguousarray(
        crow[0:D].reshape(KT, P).T).astype(np.float32)          # [P, KT]
    cbf = crow[D:3 * D].reshape(1, 2 * D).astype(ml_dtypes.bfloat16)

    wqkvT = np.ascontiguousarray(wq_eff.T).astype(ml_dtypes.bfloat16)
    wprojT = np.ascontiguousarray(w_proj.T).astype(ml_dtypes.bfloat16)
    bpack = np.concatenate([np.ones(P, np.float32),
                            b_proj.astype(np.float32)]).reshape(1, P + D)

    # If the caller's process pinned jax to cpu (common for reference
    # generation), re-discover the neuron/axon backend before the PJRT run.
    import jax
    if len(jax.devices()) < N_CORES:
        try:
            jax.config.update("jax_platforms", None)
            jax.clear_backends()
        except Exception:
            pass

    nc = _get_nc(with_bias, dbg)
    in_maps = []
    for i in range(N_CORES):
        m = {"x": np.ascontiguousarray(x[i]).astype(ml_dtypes.bfloat16),
             "wqkvT": wqkvT, "wprojT": wprojT, "bpack": bpack,
             "cq": cq_hd, "cbf": cbf}
        in_maps.append(m)

    res = run_bass_kernel_spmd(nc, in_maps, core_ids=list(range(N_CORES)),
                               trace=trace)
    out = np.stack([np.asarray(res.results[i]["out"]).astype(np.float32)
                    for i in range(N_CORES)], axis=0)
    if dbg:
        return out, res
    if trace:
        return out, res
    return out


# revision 43
# speedup vs baseline: 4.7407x; 4.7407x over previous
"""Trainium2 Bass kernel: LayerNorm -> QKV -> linear (elu+1) attention -> proj.

Data-parallel over batch: 8 batch elements, one per NeuronCore. All matmuls
in bf16 (fp32 accumulation in PSUM); LayerNorm statistics in fp32; the
projection bias is applied in fp32.

Self-contained: hardcodes shapes from the problem spec.
"""

import numpy as np
import ml_dtypes

from concourse import bass, bacc, tile, mybir
from concourse.bass import ts, ds
from concourse.bass_utils import run_bass_kernel_spmd

F32 = mybir.dt.float32
F32R = mybir.dt.float32r
BF16 = mybir.dt.bfloat16
AF = mybir.ActivationFunctionType
ALU = mybir.AluOpType

# Problem shapes
N = 4096          # tokens per batch element
D = 768           # model dim
H = 12            # heads
HD = 64           # head dim
E3 = 3 * D        # qkv width
P = 128
KT = D // P       # 6 d-tiles
NT = N // P       # 32 token tiles
CH = 8            # token chunks of 512
TPC = NT // CH    # 4 token tiles per chunk
CW = N // CH      # 512 chunk width
LN_EPS = 1e-5
EPS = 1e-6

N_CORES = 8
LDW_SKIP = True


def _build(with_qkv_bias: bool, dbg: bool = False):
    """Build the single-core program (SPMD: same NEFF on all 8 cores)."""
    nc = bacc.Bacc("TRN2", target_bir_lowering=False, debug=False,
                   num_devices=N_CORES)

    x_d = nc.dram_tensor("x", [N, D], BF16, kind="ExternalInput").ap()
    wqkvT_d = nc.dram_tensor("wqkvT", [D, E3], BF16, kind="ExternalInput").ap()
    wprojT_d = nc.dram_tensor("wprojT", [D, D], BF16, kind="ExternalInput").ap()
    bpack_d = nc.dram_tensor("bpack", [1, P + D], F32, kind="ExternalInput").ap()
    if with_qkv_bias:
        cqkv_d = nc.dram_tensor("cqkv", [1, E3], F32, kind="ExternalInput").ap()
    out_d = nc.dram_tensor("out", [N, D], BF16, kind="ExternalOutput").ap()

    from contextlib import ExitStack
    with tile.TileContext(nc) as tc, ExitStack() as stk:
        _kernel(tc, stk, nc, x_d, wqkvT_d, wprojT_d, bpack_d,
                cqkv_d if with_qkv_bias else None, out_d, dbg)

    nc.compile()
    return nc


def _kernel(tc, stk, nc, x_d, wqkvT_d, wprojT_d, bpack_d, cqkv_d, out_d,
            dbg=False):
    def dump(name, tl, shape, dtype):
        if not dbg:
            return
        d = nc.dram_tensor("dbg_" + name, shape, dtype, kind="ExternalOutput").ap()
        nc.sync.dma_start(d, tl)

    from contextlib import ExitStack
    consts = stk.enter_context(tc.tile_pool(name="consts", bufs=1))
    stk1 = stk.enter_context(ExitStack())
    ppersist = stk1.enter_context(tc.tile_pool(name="ppersist", bufs=1, space="PSUM"))

    # DMA queue split (HWDGE queues are in-order, so DMAs that wait on
    # compute must not sit ahead of dep-free ones):
    #   SP queue:  x tiles (issued 2 chunks ahead of use, so they always sit
    #              in front of the waiting bounce/transposes), the
    #              LN-dependent bounce + transposes, phase-2 zb/zr.
    #   ACT queue: weights (q columns first; w_proj deferred), out stores.
    x_prefetch = {}
    xTp = stk.enter_context(tc.tile_pool(name="xT", bufs=8))
    xpool_early = stk1.enter_context(tc.tile_pool(name="x", bufs=14))

    def load_x_chunk(c0):
        for t in range(c0 * TPC, (c0 + 1) * TPC):
            xt = xpool_early.tile([P, D], BF16)
            nc.sync.dma_start(xt[:], x_d[ts(t, P), :])
            x_prefetch[t] = xt

    wqkvT = consts.tile([P, KT, E3], BF16)
    wprojT = consts.tile([P, KT, D], BF16)
    wq_r = wqkvT_d.rearrange("(kt p) e -> p kt e", p=P)
    wp_r = wprojT_d.rearrange("(kt p) e -> p kt e", p=P)
    bproj_row = consts.tile([1, D], F32)
    nc.scalar.dma_start(bproj_row[:], bpack_d[:, P:P + D])
    load_x_chunk(0)
    load_x_chunk(1)
    nc.scalar.dma_start(wqkvT[:, :, 0:D], wq_r[:, :, 0:D])
    nc.scalar.dma_start(wqkvT[:, :, D:E3], wq_r[:, :, D:E3])

    ones_row = consts.tile([1, P], F32)
    nc.vector.memset(ones_row[:], 1.0)
    bias_sb = consts.tile([P, D], F32)

    # zero-row for psum-bank init matmuls
    zrow = consts.tile([1, 512], BF16)
    nc.vector.memset(zrow[:], 0.0)
    ones_bf = consts.tile([1, P], BF16)
    nc.vector.memset(ones_bf[:], 1.0)
    kvbd = consts.tile([P, KT, P], BF16)
    nc.vector.memset(kvbd[:], 0.0)

    # --- kv accumulator ---
    # pair p = h//2 -> cols [65p, 65p+65), head parity s=h%2 -> partitions
    # [64s, 64s+64). col 64 of each head block = k_sum.
    kv_ps = ppersist.tile([P, 6 * 65], F32)
    # Init the whole kv bank with one start=True matmul writing zeros: sets
    # every has_written bit so the 12 interleaved accumulation chains below
    # can all run with start=False. (start=True clears the *bank's* bits, so
    # per-chain start flags would clobber each other.)
    nc.tensor.matmul(kv_ps[:], ones_bf[:], zrow[:, 0:6 * 65], start=True,
                     stop=False, skip_group_check=True)

    xpool = xpool_early
    stat = stk1.enter_context(tc.tile_pool(name="stat", bufs=12))
    xhatp = stk1.enter_context(tc.tile_pool(name="xhat", bufs=4))
    kvps = stk1.enter_context(tc.tile_pool(name="kvps", bufs=2, space="PSUM"))
    qpsp = stk1.enter_context(tc.tile_pool(name="qpsp", bufs=1, space="PSUM"))
    evac = stk1.enter_context(tc.tile_pool(name="evac", bufs=4))

    qT_all = consts.tile([P, KT, N], BF16)
    dramp = stk.enter_context(tc.tile_pool(name="dram", bufs=5, space="DRAM"))

    # ============ PHASE 1: LN, transpose, k/v, kv accumulation ============
    # Processed in "pieces" of ntt token-tiles. Chunk 0 runs as two
    # 256-token halves so its LN -> bounce -> transpose fill latency is
    # halved and the PE gets its first matmuls ~15us earlier; later chunks
    # run full-width (the pipeline hides their latency).
    from contextlib import nullcontext

    def process_piece(c, tt0, ntt, q_first):
        W = ntt * P
        base = c * CW + tt0 * P
        xts = []
        mv_all = stat.tile([P, TPC, 2], F32, tag="mv")
        # For the first pipeline-filling piece, rank the LN -> xhat chain
        # ahead of neighboring pieces' stats on the in-order DVE queue.
        prio = tc.high_priority() if (c == 0 and tt0 == 0) else nullcontext()
        with prio:
            for j in range(ntt):
                xt = x_prefetch.pop(c * TPC + tt0 + j)
                xts.append(xt)
                # LayerNorm stats (fp32)
                st6 = stat.tile([P, 2, 6], F32)
                nc.vector.bn_stats(st6[:, 0], xt[:, 0:D // 2])
                nc.vector.bn_stats(st6[:, 1], xt[:, D // 2:D])
                nc.vector.bn_aggr(mv_all[:, j], st6[:])
            # batched rstd = rsqrt(var+eps): bit-trick seed + 1 Newton step
            # (seed rel err ~3.4% -> ~0.2% after one step; xhat is bf16)
            I32 = mybir.dt.int32
            veps = stat.tile([P, TPC], F32)
            nc.vector.tensor_scalar_add(veps[:, 0:ntt], mv_all[:, 0:ntt, 1],
                                        LN_EPS)
            t1 = stat.tile([P, TPC], I32, tag="rs_t1")
            nc.vector.tensor_scalar(t1[:, 0:ntt],
                                    veps[:, 0:ntt].bitcast(I32), 1, None,
                                    op0=ALU.arith_shift_right)
            rstd = stat.tile([P, TPC], F32)
            nc.vector.tensor_scalar(rstd[:, 0:ntt].bitcast(I32), t1[:, 0:ntt],
                                    -1, 0x5F3759DF, op0=ALU.mult, op1=ALU.add)
            a = stat.tile([P, TPC], F32, tag="rs_a")
            nc.vector.tensor_tensor(a[:, 0:ntt], rstd[:, 0:ntt], rstd[:, 0:ntt],
                                    ALU.mult)
            nc.vector.tensor_tensor(a[:, 0:ntt], a[:, 0:ntt], veps[:, 0:ntt],
                                    ALU.mult)
            nc.vector.tensor_scalar(a[:, 0:ntt], a[:, 0:ntt], -0.5, 1.5,
                                    op0=ALU.mult, op1=ALU.add)
            nc.vector.tensor_tensor(rstd[:, 0:ntt], rstd[:, 0:ntt], a[:, 0:ntt],
                                    ALU.mult)
            xhat = xhatp.tile([P, TPC, D], BF16)
            xh_dram = dramp.tile([CW, D], BF16)
            for j in range(ntt):
                # xhat = (x - mean) * rstd   -> bf16
                nc.vector.tensor_scalar(xhat[:, j], xts[j][:],
                                        mv_all[:, j, 0:1],
                                        rstd[:, j:j + 1],
                                        op0=ALU.subtract, op1=ALU.mult)
        # single bounce DMA per piece (one writer for the transposes)
        nc.sync.dma_start(xh_dram[0:W].rearrange("(tt p) d -> p tt d", p=P),
                          xhat[:, 0:ntt])

        # transpose the piece: [t, d] -> [d, t] via DRAM->SBUF DMA, batched
        # as two 3-kt transposes (3D out AP) to amortize the ~1.3us
        # descriptor generation per instruction on SP.
        # (all on SP: concurrent xbar transposes on both HWDGE queues
        # produce corrupted output -- verified empirically)
        xT3 = [xTp.tile([P, 3, CW], BF16, tag="xT3",
                        name=f"xT3_{c}_{tt0}_{h}") for h in range(2)]
        for h in range(2):
            nc.sync.dma_start_transpose(out=xT3[h][:, :, 0:W],
                                        in_=xh_dram[0:W, ds(h * 384, 384)])
        xT = [xT3[kt // 3][:, kt % 3, 0:W] for kt in range(KT)]

        # --- q (weight stationary, directly transposed) interleaved with
        # k/v (activation stationary) so PSUM evacuations never stall PE ---
        def q_chain(m):
            q_ps = qpsp.tile([P, 512], F32, tag="qps1")
            for kt in range(KT):
                nc.tensor.matmul(q_ps[:, 0:W], wqkvT[:, kt, ts(m, P)], xT[kt],
                                 start=(kt == 0), stop=(kt == KT - 1))
            # elu1(q) = min(exp(q),1) + relu(q); exp+relu on ACT, fuse on DVE
            et = evac.tile([P, CW], BF16, tag="elu_e")
            nc.scalar.activation(et[:, 0:W], q_ps[:, 0:W], AF.Exp)
            rt = evac.tile([P, CW], BF16, tag="elu_r")
            nc.scalar.activation(rt[:, 0:W], q_ps[:, 0:W], AF.Relu)
            nc.vector.scalar_tensor_tensor(qT_all[:, m, ds(base, W)],
                                           et[:, 0:W], 1.0, rt[:, 0:W],
                                           op0=ALU.min, op1=ALU.add)

        def kv_chain(j):
            t = c * TPC + tt0 + j
            kv3 = kvps.tile([P, 3 * 512], F32, tag="ph1ps")  # cols [768, 2304)
            for kt in range(KT):
                for jj in range(3):
                    mm = nc.tensor.matmul(
                        kv3[:, ts(jj, 512)],
                        xT[kt][:, ts(j, P)],
                        wqkvT[:, kt, ds(D + jj * 512, 512)],
                        start=(kt == 0), stop=(kt == KT - 1))
                    if jj > 0 and LDW_SKIP:
                        mm.ldweights = False  # same stationary as jj-1
            # k = elu1(cols 0:768) = min(exp, 1) + relu
            ek = evac.tile([P, D], BF16, tag="elu_ek")
            nc.scalar.activation(ek[:], kv3[:, 0:D], AF.Exp)
            rk = evac.tile([P, D], BF16, tag="elu_rk")
            nc.vector.tensor_scalar_max(rk[:], kv3[:, 0:D], 0.0)
            ktile = evac.tile([P, D], BF16, tag="ktile")
            nc.vector.scalar_tensor_tensor(ktile[:], ek[:], 1.0, rk[:],
                                           op0=ALU.min, op1=ALU.add)
            # v' = [v_h | 1] per head: [128, 12, 65]
            vtile = evac.tile([P, H, HD + 1], BF16, tag="vtile")
            nc.vector.memset(vtile[:, :, HD:HD + 1], 1.0)
            nc.scalar.activation(
                vtile[:, :, 0:HD],
                kv3[:, D:2 * D].rearrange("p (h e) -> p h e", h=H),
                AF.Copy)
            # kv accumulation: 12 heads, 2 packed per psum column block
            for h in range(H):
                p_, s_ = h // 2, h % 2
                nc.tensor.matmul(
                    kv_ps[ds(64 * s_, 64), ds(65 * p_, 65)],
                    ktile[:, ds(HD * h, HD)],
                    vtile[:, h],
                    start=False, stop=(t == NT - 1),
                    skip_group_check=True,
                    tile_position=(0, 64 * s_))

        # issue order: q0 kv0 q1 kv1 ... then remaining q chains. With
        # q_first (chunk 0, k/v weight columns still in flight) all q
        # chains run before the kv chains. kv_first (last chunk) unblocks
        # the phase boundary earlier: its kv chains complete while the PE
        # still has q chains to chew on.
        if q_first == "kv_first":
            for j in range(ntt):
                kv_chain(j)
            for m in range(KT):
                q_chain(m)
        elif q_first:
            for m in range(KT):
                q_chain(m)
            for j in range(ntt):
                kv_chain(j)
        else:
            for j in range(ntt):
                q_chain(j)
                kv_chain(j)
            for m in range(ntt, KT):
                q_chain(m)

    for c in range(CH):
        if c == 0:
            process_piece(0, 0, 2, True)
            process_piece(0, 2, 2, True)
        elif c == CH - 1:
            process_piece(c, 0, TPC, "kv_first")
        else:
            process_piece(c, 0, TPC, False)
        # x refill for chunk c+2 goes AFTER this chunk's bounce/transposes
        # in the SP queue so it cannot delay them
        if c + 2 < CH:
            load_x_chunk(c + 2)
        if c == 4:
            # w_proj is phase-2-only; issue late-ish on the ACT queue
            nc.scalar.dma_start(wprojT[:], wp_r[:])

    # ================= PHASE 1.5: kv -> sbuf, Ksel ========================
    kv_sb = consts.tile([P, 6 * 65], BF16)
    nc.scalar.activation(kv_sb[:], kv_ps[:], AF.Copy)
    # block-diagonal [128,128] attn stationaries: parity-0 head kv in the
    # (0:64,0:64) block, parity-1 in (64:128,64:128); one matmul per pair
    # covers both parities in a single 512-wide pass.
    for p_ in range(KT):
        for s_ in range(2):
            nc.scalar.activation(
                kvbd[ds(64 * s_, 64), p_, ds(64 * s_, 64)],
                kv_sb[ds(64 * s_, 64), ds(65 * p_, 64)], AF.Copy)
    dump("kv", kv_sb[:], [P, 6 * 65], BF16)
    dump("qTd", qT_all[:], [P, KT, N], BF16)
    ksel = consts.tile([P, KT, H], BF16)
    nc.vector.memset(ksel[:], 0.0)
    for kt in range(KT):
        for s_ in range(2):
            h = 2 * kt + s_
            nc.vector.tensor_copy(
                ksel[ds(64 * s_, 64), kt, h:h + 1],
                kv_sb[ds(64 * s_, 64), ds(65 * kt + 64, 1)])

    stk1.close()

    # --- broadcast b_proj to [128, D] fp32 via K=1 fp32 matmuls (placed at
    # the phase transition: off the startup critical path, and the PE is
    # otherwise underfed here) ---
    with tc.tile_pool(name="pbias", bufs=1, space="PSUM") as pbias:
        for j, w_ in ((0, 512), (1, 256)):
            bias_ps = pbias.tile([P, 512], F32)
            nc.tensor.matmul(bias_ps[:, :w_], ones_row[:],
                             bproj_row[:, ds(j * 512, w_)],
                             start=True, stop=True)
            nc.vector.tensor_copy(bias_sb[:, ds(j * 512, w_)], bias_ps[:, :w_])

    zps = stk.enter_context(tc.tile_pool(name="zps", bufs=2, space="PSUM"))
    atps = stk.enter_context(tc.tile_pool(name="atps", bufs=2, space="PSUM"))
    ops_ = stk.enter_context(tc.tile_pool(name="ops", bufs=2, space="PSUM"))
    ph2 = stk.enter_context(tc.tile_pool(name="ph2", bufs=3))
    zrpool = stk.enter_context(tc.tile_pool(name="zr", bufs=4))

    # ============ PHASE 2: z, attn out, proj ==============================
    # Per chunk: z_pre = ksel.T @ qT (PE), z = recip(z_pre + eps) (ACT),
    # z replicated to head-dim partitions via a DRAM bounce + broadcast-read
    # DMAs (stride-0 DRAM source) -- no PE/DVE cost for the replication.
    # The attention matmul runs on the UNSCALED qT (z scaling commutes with
    # the per-head contraction) and the z factor is applied during the PSUM
    # evacuation, so the attn matmuls depend only on kv_sb + qT_all and give
    # the PE real work while the first zr tiles are in flight.
    def z_chain(c):
        qT = qT_all[:, :, ts(c, CW)]
        z_ps = zps.tile([H, CW], F32)
        for kt in range(KT):
            nc.tensor.matmul(z_ps[:], ksel[:, kt], qT[:, kt],
                             start=(kt == 0), stop=(kt == KT - 1))
        zb = ph2.tile([H, CW], BF16, tag="zb")
        nc.scalar.add_instruction(mybir.InstActivation(
            name=nc.get_next_instruction_name(),
            func=AF.Reciprocal,
            ins=[nc.scalar.lower_ap(z_ps[:]),
                 mybir.ImmediateValue(dtype=F32, value=EPS),
                 mybir.ImmediateValue(dtype=F32, value=1.0),
                 mybir.ImmediateValue(dtype=F32, value=0.0)],
            outs=[nc.scalar.lower_ap(zb[:])]))
        # bounce zb to DRAM, then 2 parity broadcast-reads into [128, KT, CW]
        zb_dram = dramp.tile([H, CW], BF16)
        nc.sync.dma_start(zb_dram[:], zb[:])
        zr = zrpool.tile([P, KT, CW], BF16, tag="zr")
        zb_r = zb_dram.rearrange("(pp s) w -> s pp w", s=2)
        for s_ in range(2):
            nc.sync.dma_start(
                zr[ds(64 * s_, 64), :, :],
                zb_r[s_:s_ + 1].broadcast_to([64, KT, CW]))
        return zr

    def attn_mms(c, fused):
        """attn_T[e, t] per head pair on unscaled qT; one block-diagonal
        [128,128] stationary covers both parities in a single pass.
        fused=True: z-scale applied during the DVE evacuation (waits on zr).
        fused=False: plain ACT-copy evac (no zr dep) -- the caller scales in
        place later."""
        qT = qT_all[:, :, ts(c, CW)]
        attnT = ph2.tile([P, KT, CW], BF16, tag="attnT")
        for p_ in range(KT):
            at_ps = atps.tile([P, CW], F32)
            nc.tensor.matmul(at_ps[:], kvbd[:, p_], qT[:, p_],
                             start=True, stop=True)
            if fused:
                nc.vector.tensor_mul(attnT[:, p_], at_ps[:],
                                     zr_tiles[c][:, p_])
            else:
                nc.scalar.activation(attnT[:, p_], at_ps[:], AF.Copy)
        return attnT

    # z three chunks ahead (the DRAM round trip needs ~8us of cover);
    # attn two chunks ahead (dep-free filler for the transition). The attn
    # matmuls need only kv_sb (ready before ksel), so they issue first.
    zr_tiles = {}
    attn_tiles = {}
    attn_tiles[0] = attn_mms(0, fused=False)
    attn_tiles[1] = attn_mms(1, fused=False)
    for c0 in range(3):
        zr_tiles[c0] = z_chain(c0)


    for c in range(CH):
        attnT = attn_tiles.pop(c)
        zr = zr_tiles.pop(c)
        if c < 2:
            # late in-place z-scale (zr was still in flight at issue time)
            for kt in range(KT):
                nc.vector.tensor_mul(attnT[:, kt], attnT[:, kt], zr[:, kt])
        if c == 0:
            dump("zr0", zr[:], [P, KT, CW], BF16)
            dump("attnT0", attnT[:], [P, KT, CW], BF16)

        # proj: out[t, e] = sum_d attnT[d, t] * wprojT[d, e]  (+ bias)
        osb = ph2.tile([P, TPC, D], BF16, tag="osb")
        for tt in range(TPC):
            o_ps = ops_.tile([P, D], F32)
            for kt in range(KT):
                for j, w_ in ((0, 512), (1, 256)):
                    mm = nc.tensor.matmul(
                        o_ps[:, ds(j * 512, w_)],
                        attnT[:, kt, ts(tt, P)],
                        wprojT[:, kt, ds(j * 512, w_)],
                        start=(kt == 0), stop=(kt == KT - 1))
                    if j > 0 and LDW_SKIP:
                        mm.ldweights = False  # same stationary as j-1
            nc.vector.tensor_tensor(osb[:, tt], o_ps[:], bias_sb[:], ALU.add)
        nc.scalar.dma_start(
            out_d[ts(c, CW), :].rearrange("(tt p) d -> p tt d", p=P), osb[:])

        if c + 3 < CH:
            zr_tiles[c + 3] = z_chain(c + 3)
        if c + 2 < CH:
            attn_tiles[c + 2] = attn_mms(c + 2, fused=True)


_CACHE = {}


def _get_nc(with_qkv_bias: bool, dbg: bool = False):
    key = ("nc", with_qkv_bias, dbg)
    if key not in _CACHE:
        _CACHE[key] = _build(with_qkv_bias, dbg)
    return _CACHE[key]


def kernel(x, ln_gamma, ln_beta, w_qkv, w_proj, b_proj, trace=False, dbg=False):
    x = np.asarray(x, dtype=np.float32)
    ln_gamma = np.asarray(ln_gamma, dtype=np.float32)
    ln_beta = np.asarray(ln_beta, dtype=np.float32)
    w_qkv = np.asarray(w_qkv, dtype=np.float32)
    w_proj = np.asarray(w_proj, dtype=np.float32)
    b_proj = np.asarray(b_proj, dtype=np.float32)
    bsz = x.shape[0]
    assert x.shape == (bsz, N, D) and bsz == N_CORES

    # Fold LN affine into the qkv projection (exact algebra):
    #   y = xhat*gamma + beta  =>  qkv = xhat @ (gamma*W)^T + W@beta
    wq_eff = (w_qkv * ln_gamma[None, :])          # [E3, D]
    cqkv = w_qkv @ ln_beta                        # [E3]
    with_bias = bool(np.any(cqkv))
    if with_bias:
        raise NotImplementedError(
            "nonzero W@beta path not wired into the device kernel")

    wqkvT = np.ascontiguousarray(wq_eff.T).astype(ml_dtypes.bfloat16)
    wprojT = np.ascontiguousarray(w_proj.T).astype(ml_dtypes.bfloat16)
    bpack = np.concatenate([np.ones(P, np.float32),
                            b_proj.astype(np.float32)]).reshape(1, P + D)

    # If the caller's process pinned jax to cpu (common for reference
    # generation), re-discover the neuron/axon backend before the PJRT run.
    import jax
    if len(jax.devices()) < N_CORES:
        try:
            jax.config.update("jax_platforms", None)
            jax.clear_backends()
        except Exception:
            pass

    nc = _get_nc(with_bias, dbg)
    in_maps = []
    for i in range(N_CORES):
        m = {"x": np.ascontiguousarray(x[i]).astype(ml_dtypes.bfloat16),
             "wqkvT": wqkvT, "wprojT": wprojT, "bpack": bpack}
        in_maps.append(m)

    res = run_bass_kernel_spmd(nc, in_maps, core_ids=list(range(N_CORES)),
                               trace=trace)
    out = np.stack([np.asarray(res.results[i]["out"]).astype(np.float32)
                    for i in range(N_CORES)], axis=0)
    if dbg:
        return out, res
    if trace:
        return out, res
    return out

